# revision 40
# baseline (speedup 1.0000x reference)
"""Multi-head attention (B=16, L=S=1024, D=P=512, H=8) on 8 TRN2 NeuronCores.

Strategy: pure data parallelism over the batch — each core computes the full
attention block for 2 batch elements.  Activations are fed to the device
pre-transposed ([D, L] instead of [L, D]) so every GEMM contracts over the
partition dimension with no on-chip transposes:

  per batch element b (all on one core):
    QT[P,L] = Wq.T @ qT + bq   bf16 inputs, fp16 result
    KT[P,S] = Wk.T @ kT + bk   (fp16 so the K=64-contraction scores matmuls
                                run 1 cycle/col; f32r ran at 2 cycles/col)
    V [S,P] = vT.T @ Wv + bv   bf16 inputs, fp16 result in 128-wide head
                               blocks [1.0, 0*63, V_h] so the A@V matmul
                               emits softmax sums at psum partition 0 and the
                               head output at partitions 64..127 (both legal
                               PSUM offsets; the fast-recip custom-DVE op
                               reads sums straight from PSUM partition 0)
    per head h (E=64), software-pipelined two (h, L-chunk) chunks deep:
      expT[S,L] = exp(scale * K_h @ Q_h^T)
          S-chunks 0-5 on ACT (table exp, fp16 out); chunks 6-7 on the DVE
          as a Schraudolph bit-trick (int16(A*score+B) read back as fp16,
          ~3% sawtooth; softmax normalization cancels uniform scale error) —
          the ACT engine alone (1 col/cycle @1.2GHz) cannot keep up with the
          PE, and GPSIMD has no PSUM access so DVE is the only helper
      OT_h[E,L] = V128_h.T @ expT; OT rows normalized by the fast recip
          (GpSimd broadcasts the [1,512] recip row to 64 partitions)
    out[L,D] = OT.T-contraction with Wo + bo, emitted per L-half as soon as
          that half's last head is normalized (shrinks the tail)

Scheduling notes (measured on HW):
  - psum pools: scores+projections share one 3-buffer [128,1024] pool slot
    set; A@V + V-proj share a 2-buffer [128,512] pool (8 banks total).  The
    third scores buffer removes exp->matmul backpressure stalls.
  - The first two chunks' scores are issued before the V projection so the
    exp pipeline is full when the attention loop starts.
  - Weights/activations load as per-dt tiles: dependency tracking is
    tile-granular, so fused tiles made the first matmul wait on all 4 DMAs.
  - fp8 DoubleRow was measured a wash: it does halve A@V streaming (2
    moving cols/cycle), but the denominator matmuls it displaces (ones
    column no longer fits the M<=64 stationary) cost exactly the saving,
    and per-element fp8 error pushed rel-err to ~1.8e-2 vs the 2e-2 gate.

Roofline: the PE streams ~391k columns/core at 1 col/cycle @2.4GHz = 165us;
this kernel measures ~207.5us HW exec (~89% PE occupancy incl. ~11us fixed
startup and ~6us drain/epilogue), rel err ~3.9e-3 (gate 2e-2).  The TRN2
clock throttles ~17% after sustained back-to-back runs; timings above are
cold-chip numbers.
"""

import numpy as np

B, L, S, D, P, H, E = 16, 1024, 1024, 512, 512, 8, 64
NCORES = 8
BPC = B // NCORES  # batch elements per core
SCALE = 1.0 / float(np.sqrt(E))
WSCALE = 16.0  # host premultiplier on Wq/Wk so fp8e4 stays in normal range

# Schraudolph exp, fp16 flavor: the DVE computes v = A*score + B in fp32 and
# converts to int16; the bit pattern read back as fp16 is ~exp(scale*score)
# with ~3% sawtooth error (softmax normalization cancels any uniform scale
# error, only the sawtooth shape survives).
SCHRAUD_A = float(2**10 / np.log(2)) * SCALE  # folds the 1/sqrt(E) scale
SCHRAUD_B = float((15 - 0.043677448) * 2**10)  # 15 = fp16 exponent bias
DVE_STS = frozenset((6, 7))  # which of the 8 S-chunks per (h,lc) go to DVE


def _exp_plan(sp):
    """Per score-psum pair sp (sts 2sp, 2sp+1): list of (j0, j1, on_dve) ops."""
    a = (sp * 2) in DVE_STS
    b = (sp * 2 + 1) in DVE_STS
    if a == b:
        return [(0, 2, a)]
    return [(0, 1, a), (1, 2, b)]

_CACHE = {}
LAST_RESULTS = None  # stashed BassKernelResults for test harness introspection


def _build():
    """Build (once) the Bass program executed identically on all 8 cores."""
    if "nc" in _CACHE:
        return _CACHE["nc"]

    from contextlib import ExitStack

    import concourse.bass as bass
    import concourse.mybir as mybir
    import concourse.tile as tile
    from concourse import bacc

    f32 = mybir.dt.float32
    f32r = mybir.dt.float32r
    f16 = mybir.dt.float16
    i16 = mybir.dt.int16
    bf16 = mybir.dt.bfloat16
    fp8 = mybir.dt.float8e4
    AF = mybir.ActivationFunctionType
    ALU = mybir.AluOpType
    DR = mybir.MatmulPerfMode.DoubleRow

    nc = bacc.Bacc("TRN2", target_bir_lowering=False, debug=False)

    qT = nc.dram_tensor("qT", [BPC, D, L], bf16, kind="ExternalInput").ap()
    kT = nc.dram_tensor("kT", [BPC, D, S], bf16, kind="ExternalInput").ap()
    vT = nc.dram_tensor("vT", [BPC, D, S], bf16, kind="ExternalInput").ap()
    Wq = nc.dram_tensor("Wq", [D, P], bf16, kind="ExternalInput").ap()
    Wk = nc.dram_tensor("Wk", [D, P], bf16, kind="ExternalInput").ap()
    Wv = nc.dram_tensor("Wv", [D, P], bf16, kind="ExternalInput").ap()
    Wo = nc.dram_tensor("Wo", [P, D], f32, kind="ExternalInput").ap()
    bq_col = nc.dram_tensor("bq_col", [128, 4], f32, kind="ExternalInput").ap()
    bk_col = nc.dram_tensor("bk_col", [128, 4], f32, kind="ExternalInput").ap()
    bv_row = nc.dram_tensor("bv_row", [P], f32, kind="ExternalInput").ap()
    bo_row = nc.dram_tensor("bo_row", [D], f32, kind="ExternalInput").ap()
    ones_in = nc.dram_tensor("ones_in", [128, 128], f32, kind="ExternalInput").ap()
    out = nc.dram_tensor("out", [BPC, L, D], f32, kind="ExternalOutput").ap()

    def bcast_ap(src, n=128):
        # [N] DRAM vector (or [1, N] SBUF row) -> [n, N] partition-broadcast AP
        return bass.AP(tensor=src.tensor, offset=src.offset, ap=[[0, n]] + src.ap[-1:])

    with tile.TileContext(nc) as tc, ExitStack() as ctx:
        consts = ctx.enter_context(tc.tile_pool(name="consts", bufs=1))
        xT_pool = ctx.enter_context(tc.tile_pool(name="xT", bufs=2))
        acts = ctx.enter_context(tc.tile_pool(name="acts", bufs=1))
        exp_pool = ctx.enter_context(tc.tile_pool(name="exp", bufs=3))
        small = ctx.enter_context(tc.tile_pool(name="small", bufs=2))
        out_pool = ctx.enter_context(tc.tile_pool(name="outp", bufs=3))
        psum = ctx.enter_context(tc.tile_pool(name="psum", bufs=3, space="PSUM"))
        psum_ot = ctx.enter_context(tc.tile_pool(name="psum_ot", bufs=2, space="PSUM"))

        # ---- constants: weights [128, dtile, N] with contraction dim on partitions.
        # DMA issue order is interleaved with the first batch's activation loads
        # below so the first projection matmul isn't queued behind the weights.
        # per-dt tiles: tile-granular dependency tracking means a matmul on
        # dt=0 would otherwise wait for all four dt DMAs of a fused tile
        Wq_sb = [consts.tile([128, P], bf16, tag=f"Wq{dt}", name=f"Wq{dt}") for dt in range(4)]
        Wk_sb = [consts.tile([128, P], bf16, tag=f"Wk{dt}", name=f"Wk{dt}") for dt in range(4)]
        Wv_sb = [consts.tile([128, P], bf16, tag=f"Wv{dt}", name=f"Wv{dt}") for dt in range(4)]
        Wo_sb = [consts.tile([128, D], f32r, tag=f"Wo{dt}", name=f"Wo{dt}") for dt in range(4)]
        bq_sb = consts.tile([128, 4], f32, tag="bq")
        bk_sb = consts.tile([128, 4], f32, tag="bk")
        bv_sb = consts.tile([128, P], f32, tag="bv")
        bo_sb = consts.tile([128, D], f32, tag="bo")

        # V in 128-wide head blocks: col h*128 = 1.0, cols +1..63 = 0, cols
        # +64..127 = head h of V.  The OT matmul's [128,128] stationary then
        # emits the softmax denominator at psum PARTITION 0 (ones column) and
        # the head output at partitions 64..127 -- both PSUM-aligned offsets,
        # so the fast-recip custom-DVE op reads the sums straight from PSUM
        # (nonzero psum partition offsets trip a HW bug in custom-DVE ops).
        V_sb = consts.tile([128, 8, 8 * 128], f16, tag="V")  # [S-part, stile, 1024]
        Vv = V_sb.rearrange("p s (h e) -> p s h e", e=128)
        nc.vector.memset(Vv[:, :, :, 1:64], 0.0)
        nc.vector.memset(Vv[:, :, :, 0:1], 1.0)

        def load_xT(src, b, name, dtype):
            # per-dt tiles + DMAs so each projection matmul waits only its dt
            ts = [xT_pool.tile([128, L], dtype, tag=f"{name}{dt}", name=f"{name}{dt}")
                  for dt in range(4)]
            view = src[b].rearrange("(t p) l -> p t l", p=128)
            for dt in range(4):
                nc.sync.dma_start(out=ts[dt], in_=view[:, dt, :])
            return ts

        def load_w(W_sb, Wsrc, dtype):
            view = Wsrc.rearrange("(t p) n -> p t n", p=128)
            if dtype == f32r:
                view = view.bitcast(f32r)
            for dt in range(4):
                nc.sync.dma_start(out=W_sb[dt], in_=view[:, dt, :])

        # The first psum group consumes (Wq[dt], qT[dt]) in dt order: issue the
        # DMAs in exactly that order, alternating across the sync and gpsimd
        # queues so transfers overlap.
        Wq_view = Wq.rearrange("(t p) n -> p t n", p=128)
        qT_view = qT[0].rearrange("(t p) l -> p t l", p=128)
        qT0_sb = [xT_pool.tile([128, L], bf16, tag=f"qT_sb{dt}", name=f"qT0_{dt}")
                  for dt in range(4)]
        nc.sync.dma_start(out=Wq_sb[0], in_=Wq_view[:, 0, :])
        nc.gpsimd.dma_start(out=qT0_sb[0], in_=qT_view[:, 0, :])
        nc.sync.dma_start(out=Wq_sb[1], in_=Wq_view[:, 1, :])
        nc.gpsimd.dma_start(out=qT0_sb[1], in_=qT_view[:, 1, :])
        nc.sync.dma_start(out=Wq_sb[2], in_=Wq_view[:, 2, :])
        nc.gpsimd.dma_start(out=qT0_sb[2], in_=qT_view[:, 2, :])
        nc.sync.dma_start(out=Wq_sb[3], in_=Wq_view[:, 3, :])
        nc.gpsimd.dma_start(out=qT0_sb[3], in_=qT_view[:, 3, :])
        first = {"qT_sb": qT0_sb}
        nc.sync.dma_start(out=bq_sb, in_=bq_col)
        load_w(Wk_sb, Wk, bf16)
        nc.sync.dma_start(out=bk_sb, in_=bk_col)
        first["kT_sb"] = load_xT(kT, 0, "kT_sb", bf16)
        load_w(Wv_sb, Wv, bf16)
        nc.gpsimd.dma_start(out=bv_sb, in_=bcast_ap(bv_row))
        first["vT_sb"] = load_xT(vT, 0, "vT_sb", bf16)
        load_w(Wo_sb, Wo, f32r)
        nc.gpsimd.dma_start(out=bo_sb, in_=bcast_ap(bo_row))

        for b in range(BPC):
            if b == 0:
                qT_sb, kT_sb, vT_sb = first["qT_sb"], first["kT_sb"], first["vT_sb"]
            else:
                qT_sb = load_xT(qT, b, "qT_sb", bf16)
                kT_sb = load_xT(kT, b, "kT_sb", bf16)
                vT_sb = load_xT(vT, b, "vT_sb", bf16)

            QT_sb = acts.tile([128, 4, L], f16, tag="QT")  # [P-part, ptile, L]
            KT_sb = acts.tile([128, 4, S], f16, tag="KT")
            OT_sb = acts.tile([128, 4, L], f32r, tag="OT")  # [P-part, ptile, L]

            # ---- QT / KT projections, fp8 DoubleRow:
            # psum[p, l] = sum_d W[d, p] * xT[d, l]; W carries a x16 host
            # prescale (fp8e4 range), undone in the bias tensor_scalar.
            for W_sb, b_sb, X_sb, Y_sb in (
                (Wq_sb, bq_sb, qT_sb, QT_sb),
                (Wk_sb, bk_sb, kT_sb, KT_sb),
            ):
                for pt in range(4):
                    for lc in range(2):
                        ps = psum.tile([128, 1024], f32, tag="scores", name="ps")[:, 0:512]
                        for dt in range(4):
                            nc.tensor.matmul(
                                ps,
                                W_sb[dt][:, pt * 128:(pt + 1) * 128],
                                X_sb[dt][:, lc * 512:(lc + 1) * 512],
                                start=(dt == 0),
                                stop=(dt == 3),
                            )
                        nc.vector.tensor_scalar_add(
                            Y_sb[:, pt, lc * 512:(lc + 1) * 512], ps, b_sb[:, pt:pt + 1]
                        )

            # ---- attention, software-pipelined one (head, L-chunk) deep so the
            # PE runs scores(c) while ACT/GpSimd still exponentiate chunk c-1.
            def emit_scores_half(h, lc, expT_c, sps):
                pt_h, po_h = h // 2, (h % 2) * 64
                lsl = slice(lc * 512, (lc + 1) * 512)
                for sp in sps:
                    ps_s = psum.tile([128, 1024], f32, tag="scores", name="ps_s")
                    for j in range(2):
                        st = sp * 2 + j
                        nc.tensor.matmul(
                            ps_s[:, j * 512:(j + 1) * 512],
                            KT_sb[po_h:po_h + 64, pt_h, st * 128:(st + 1) * 128],
                            QT_sb[po_h:po_h + 64, pt_h, lsl],
                            start=True,
                            stop=True,
                        )
                    # exp split: ACT does sts 0-4 (table exp, consumed first
                    # by the OT matmuls), DVE does sts 5-7 (Schraudolph: int16
                    # bits <- scores*(A*scale)+B, read back as fp16 exp; DVE
                    # not GpSimd because GPSIMD has no PSUM access on HW).
                    psv = ps_s.rearrange("p (a b) -> p a b", b=512)
                    for j0, j1, on_dve in _exp_plan(sp):
                        if on_dve:
                            nc.vector.tensor_scalar(
                                out=expT_c[:, sp * 2 + j0:sp * 2 + j1, :].bitcast(i16),
                                in0=psv[:, j0:j1, :],
                                scalar1=SCHRAUD_A,
                                scalar2=SCHRAUD_B,
                                op0=ALU.mult,
                                op1=ALU.add,
                            )
                        else:
                            nc.scalar.activation(
                                out=expT_c[:, sp * 2 + j0:sp * 2 + j1, :],
                                in_=psv[:, j0:j1, :],
                                func=AF.Exp,
                                scale=SCALE,
                            )

            def emit_ot(h, lc, expT_c, ps_o):
                for st in range(8):
                    nc.tensor.matmul(
                        ps_o,
                        V_sb[:, st, h * 128:(h + 1) * 128],
                        expT_c[:, st, :],
                        start=(st == 0),
                        stop=(st == 7),
                    )

            def emit_norm(h, lc, ps_o):
                pt_h, po_h = h // 2, (h % 2) * 64
                lsl = slice(lc * 512, (lc + 1) * 512)
                recip_sb = small.tile([1, 512], f32, tag="recip", name="recip_sb")
                nc.vector.reciprocal_approx_fast(out=recip_sb, in_=ps_o[0:1, :])
                rep_sb = small.tile([64, 512], f32, tag="rep", name="rep_sb")
                nc.gpsimd.partition_broadcast(rep_sb, recip_sb, channels=64)
                nc.vector.tensor_mul(
                    OT_sb[po_h:po_h + 64, pt_h, lsl], ps_o[64:128, :], rep_sb
                )


            def emit_chunk(h, lc):
                expT_c = exp_pool.tile([128, 8, 512], f16, tag="expT", name="expT_c")
                emit_scores_half(h, lc, expT_c, (0, 1))
                emit_scores_half(h, lc, expT_c, (2, 3))
                return expT_c

            # prime the exp pipeline: the first two chunks' scores issue
            # before the V projection (whose psum comes from the ot pool, so
            # it does not couple to these tiles' exp completions); their
            # exponentials finish while the PE runs the V projection.
            primed = [(h, 0, emit_chunk(h, 0)) for h in range(2)]

            # ---- V projection (bf16): psum[s, p] = sum_d vT[d, s] * Wv[d, p]
            for st in range(8):
                ps = psum_ot.tile([128, 512], f32, tag="ot", name="ps")
                for dt in range(4):
                    nc.tensor.matmul(
                        ps,
                        vT_sb[dt][:, st * 128:(st + 1) * 128],
                        Wv_sb[dt],
                        start=(dt == 0),
                        stop=(dt == 3),
                    )
                nc.vector.tensor_add(
                    Vv[:, st, :, 64:128],
                    ps.rearrange("p (h e) -> p h e", e=64),
                    bv_sb.rearrange("p (h e) -> p h e", e=64),
                )

            def emit_out_proj_half(lc):
                # out projection for l rows lc*512..lc*512+511 (needs all heads
                # of that L-half in OT_sb): psum[l, d] = sum_p OT[p,l]*Wo[p,d]
                for lt in range(lc * 4, lc * 4 + 4):
                    ps = psum.tile([128, 1024], f32, tag="scores", name="ps")[:, 0:512]
                    for pt in range(4):
                        nc.tensor.matmul(
                            ps,
                            OT_sb[:, pt, lt * 128:(lt + 1) * 128],
                            Wo_sb[pt],
                            start=(pt == 0),
                            stop=(pt == 3),
                        )
                    o_sb = out_pool.tile([128, 512], f32, tag="osb")
                    nc.vector.tensor_add(o_sb, ps, bo_sb)
                    nc.sync.dma_start(out=out[b, lt * 128:(lt + 1) * 128, :], in_=o_sb)

            def pop_pending():
                ph, plc, pexp = pending.pop(0)
                ps_o = psum_ot.tile([128, 512], f32, tag="ot", name="ps_o")
                emit_ot(ph, plc, pexp, ps_o)
                emit_norm(ph, plc, ps_o)
                if (ph, plc) == (H - 1, 0):
                    emit_out_proj_half(0)  # all lc=0 heads normalized
                return plc

            pending = list(primed)
            for h in range(H):
                for lc in range(2):
                    if lc == 0 and h < len(primed):
                        continue  # scores already issued before the V proj
                    pending.append((h, lc, emit_chunk(h, lc)))
                    if len(pending) > 2:  # 2-deep stagger: OT runs two chunks behind
                        pop_pending()
            while pending:
                pop_pending()
            emit_out_proj_half(1)

    nc.compile()
    _CACHE["nc"] = nc
    return nc


def _in_maps(inputs):
    import ml_dtypes

    b16 = ml_dtypes.bfloat16
    f = lambda a: np.ascontiguousarray(np.asarray(a, dtype=np.float32))
    g = lambda a: np.ascontiguousarray(np.asarray(a, dtype=np.float32).astype(b16))
    queries, keys, values = f(inputs["queries"]), f(inputs["keys"]), f(inputs["values"])
    Wq, Wk, Wv, Wo = f(inputs["Wq"]), f(inputs["Wk"]), f(inputs["Wv"]), f(inputs["Wo"])
    bq, bk, bv, bo = f(inputs["bq"]), f(inputs["bk"]), f(inputs["bv"]), f(inputs["bo"])
    shared = {
        "Wq": g(Wq), "Wk": g(Wk), "Wv": g(Wv), "Wo": Wo,
        "bq_col": np.ascontiguousarray(bq.reshape(4, 128).T),
        "bk_col": np.ascontiguousarray(bk.reshape(4, 128).T),
        "bv_row": bv, "bo_row": bo,
        "ones_in": np.ones((128, 128), np.float32),
    }
    maps = []
    for c in range(NCORES):
        sl = slice(BPC * c, BPC * (c + 1))
        maps.append({
            "qT": np.ascontiguousarray(queries[sl].transpose(0, 2, 1).astype(b16)),
            "kT": np.ascontiguousarray(keys[sl].transpose(0, 2, 1).astype(b16)),
            "vT": np.ascontiguousarray(values[sl].transpose(0, 2, 1).astype(b16)),
            **shared,
        })
    return maps


def kernel(**inputs) -> np.ndarray:
    global LAST_RESULTS
    from concourse import bass_utils

    nc = _build()
    maps = _in_maps(inputs)
    res = bass_utils.run_bass_kernel_spmd(nc, maps, core_ids=list(range(NCORES)))
    LAST_RESULTS = res
    return np.concatenate([res.results[c]["out"] for c in range(NCORES)], axis=0)
